# revision 12
# baseline (speedup 1.0000x reference)
import sys

sys.path.insert(0, "/opt/trn_rl_repo")

import numpy as np

import concourse.bacc as bacc
import concourse.mybir as mybir
import concourse.tile as tile
from concourse.bass_utils import run_bass_kernel_spmd
from concourse.masks import make_identity

# Problem constants (nn_AMMConv2d: 3x3 conv via product quantization, STE forward)
NC, K, SUB = 16, 16, 72
CIN, COUT = 128, 256
H = W = 56
B = 8
PW = W + 2             # padded width 58
NT = 128               # positions per tile (flattened padded coords)
P0 = PW + 1            # first valid flat position (row 1, col 1) = 59
PLAST = W * PW + W     # last valid flat position = 3304
NTILES = (PLAST - P0 + 1 + NT - 1) // NT   # 26
XPL = 3456             # 1 guard col + 58*58 padded image + tail guard
CK = NC * K            # 256
# tap offsets in flattened padded coords (kh-1)*PW + (kw-1)
TAPS = [(kh - 1) * PW + (kw - 1) for kh in range(3) for kw in range(3)]
# xp DMA chunks in columns (first small for fast pipeline start)
XCHUNKS = [256, 512, 768, 896, 1024]
# output tile groups (start, size); tail groups shrink to drain faster
_OG = [(0, 4), (4, 4), (8, 4), (12, 4), (16, 4), (20, 2), (22, 2), (24, 1), (25, 1)]
OGROUPS = {}
for _s, _n in _OG:
    for _t in range(_s, _s + _n):
        OGROUPS[_t] = (_s, _n)
OGRP = 4
NWARM = 12             # PE warmup transposes (p-state ramp while DMAs land)

F32 = mybir.dt.float32
F32R = mybir.dt.float32r
F16 = mybir.dt.float16


def build():
    nc = bacc.Bacc("TRN2", target_bir_lowering=False, debug=False)

    xp_ext = nc.declare_dram_parameter("xp", [CIN, XPL], F32R, isOutput=False)
    cmm_ext = nc.declare_dram_parameter("cmm", [CIN, 9 * CK], F32R, isOutput=False)
    c2g_ext = nc.declare_dram_parameter("c2g", [CIN, CK], F32, isOutput=False)
    lut_ext = nc.declare_dram_parameter("lut", [CIN, 2 * COUT], F32R, isOutput=False)
    idn_ext = nc.declare_dram_parameter("idn", [NT, NT], F32R, isOutput=False)
    # partition-major output: out[p, t, o]; host untangles valid rows
    out_ext = nc.declare_dram_parameter(
        "out", [CIN, NTILES * COUT], F16, isOutput=True
    )
    out3 = out_ext[:].rearrange("p (t o) -> p t o", t=NTILES)

    with tile.TileContext(nc) as tc:
        with (
            tc.tile_pool(name="const", bufs=1) as const_pool,
            tc.tile_pool(name="work", bufs=4) as work,
            tc.tile_pool(name="obuf", bufs=2) as obuf,
            tc.tile_pool(name="spsum", bufs=3, space="PSUM") as spsum,
            tc.tile_pool(name="tpsum", bufs=2, space="PSUM") as tpsum,
            tc.tile_pool(name="wpsum", bufs=1, space="PSUM") as wpsum,
            tc.tile_pool(name="opsum", bufs=2, space="PSUM") as opsum,
        ):
            xp = const_pool.tile([CIN, XPL], F32R)
            cmm = const_pool.tile([CIN, 9, CK], F32R)
            c2g = const_pool.tile([CIN, CK], F32)
            lut = const_pool.tile([CIN, 2, COUT], F32R)

            # identity first so PE warmup can start immediately
            ident = const_pool.tile([NT, NT], F32R)

            # sync queue: xp column chunks, small first
            c0 = 0
            for ncols in XCHUNKS:
                c1 = min(c0 + ncols, XPL)
                nc.sync.dma_start(xp[:, c0:c1], xp_ext[:, c0:c1])
                c0 = c1
            # centroid matrices: tap pairs split across two queues for
            # parallel issue + transfer (per-tap sems stagger availability)
            nc.scalar.dma_start(ident[:], idn_ext[:])
            cmm2 = cmm[:].rearrange("p a b -> p (a b)")
            for lo, hi in ((0, 1), (1, 3), (3, 5)):
                nc.scalar.dma_start(
                    cmm2[:, lo * CK : hi * CK], cmm_ext[:, lo * CK : hi * CK]
                )
            for lo, hi in ((5, 7), (7, 9)):
                nc.gpsimd.dma_start(
                    cmm2[:, lo * CK : hi * CK], cmm_ext[:, lo * CK : hi * CK]
                )
            # epilogue consts (needed ~1 tile in)
            nc.gpsimd.dma_start(c2g[:], c2g_ext[:])
            nc.gpsimd.dma_start(
                lut[:].rearrange("p a b -> p (a b)"), lut_ext[:]
            )

            # PE warmup: ramp the tensor-engine p-state while DMAs land
            warm_ps = wpsum.tile([NT, NT], F32R, tag="warm", name="warm")
            for _ in range(NWARM):
                nc.tensor.transpose(warm_ps[:], ident[:], ident[:])

            # ---------- main loop: 3-deep software pipeline ----------
            def emit_scores(t):
                base = 1 + P0 + t * NT  # guard col + flat window start
                s_ps = spsum.tile([NT, CK], F32, tag="scores", name="s_ps")
                for kk in range(9):
                    off = TAPS[kk]
                    nc.tensor.matmul(
                        s_ps[:],
                        xp[:, base + off : base + off + NT],
                        cmm[:, kk, :],
                        start=(kk == 0),
                        stop=(kk == 8),
                    )
                return s_ps

            def emit_epi_a(t, s_ps):
                # g = xc - c2/2 (gpsimd); row-max per codebook + one-hot (DVE)
                g = work.tile([NT, CK], F32, tag="g", name="g")
                nc.vector.tensor_tensor(
                    g[:], s_ps[:], c2g[:], mybir.AluOpType.add
                )
                g3 = g[:].rearrange("p (c k) -> p c k", c=NC)
                gmax = work.tile([NT, NC], F32, tag="gmax", name="gmax")
                nc.vector.tensor_reduce(
                    gmax[:], g3, axis=mybir.AxisListType.X, op=mybir.AluOpType.max
                )
                mask = work.tile([NT, CK], F32R, tag="mask", name="mask")
                nc.vector.tensor_tensor(
                    mask[:].rearrange("p (c k) -> p c k", c=NC),
                    g3,
                    gmax[:].unsqueeze(2).broadcast_to([NT, NC, K]),
                    mybir.AluOpType.is_equal,
                )
                return mask

            def emit_epi_b(t, mask, o_sb):
                mt_ps = tpsum.tile([CIN, 2 * NT], F32R, tag="mt", name="mt_ps")
                for j in range(2):
                    nc.tensor.transpose(
                        mt_ps[:, j * NT : (j + 1) * NT],
                        mask[:, j * CIN : (j + 1) * CIN],
                        ident[:],
                    )
                mT = work.tile([CIN, 2 * NT], F32R, tag="mT", name="mT")
                nc.scalar.activation(
                    mT[:], mt_ps[:], mybir.ActivationFunctionType.Copy
                )
                o_ps = opsum.tile([NT, COUT], F32, tag="out", name="o_ps")
                for j in range(2):
                    nc.tensor.matmul(
                        o_ps[:],
                        mT[:, j * NT : (j + 1) * NT],
                        lut[:, j, :],
                        start=(j == 0),
                        stop=(j == 1),
                    )
                t0, gsz = OGROUPS[t]
                slot = t - t0
                nc.scalar.activation(
                    o_sb[:, slot * COUT : (slot + 1) * COUT],
                    o_ps[:],
                    mybir.ActivationFunctionType.Copy,
                )
                if slot == gsz - 1:
                    nc.sync.dma_start(
                        out3[:, t0 : t0 + gsz, :],
                        o_sb[:, : gsz * COUT].rearrange(
                            "p (a b) -> p a b", a=gsz
                        ),
                    )

            def get_osb(t):
                if OGROUPS[t][0] == t:
                    get_osb.cur = obuf.tile(
                        [NT, OGRP * COUT], F16, tag="osb", name="o_sb"
                    )
                return get_osb.cur

            stage_a = []  # (t, s_ps)
            stage_b = []  # (t, mask)
            for t in range(NTILES):
                stage_a.append((t, emit_scores(t)))
                if len(stage_a) > 1:
                    ta, s_ps = stage_a.pop(0)
                    stage_b.append((ta, emit_epi_a(ta, s_ps)))
                if len(stage_b) > 1:
                    tb, mask = stage_b.pop(0)
                    emit_epi_b(tb, mask, get_osb(tb))
            while stage_a:
                ta, s_ps = stage_a.pop(0)
                stage_b.append((ta, emit_epi_a(ta, s_ps)))
            while stage_b:
                tb, mask = stage_b.pop(0)
                emit_epi_b(tb, mask, get_osb(tb))

    nc.compile()
    return nc


def prep_consts(centroids, weight, bias):
    """Host-side constant packing (exact f32/f16; no device prologue math)."""
    centroids = np.asarray(centroids, dtype=np.float32)
    weight = np.asarray(weight, dtype=np.float32)
    bias = np.asarray(bias, dtype=np.float32)

    # cmm[8c+a, kk*CK + c*K + k] = centroids[c, k, a*9 + kk]
    cents_mm = np.zeros((9, CIN, CK), dtype=np.float32)
    cs = centroids.reshape(NC, K, 8, 9)  # s = a*9 + kk
    for c in range(NC):
        for a in range(8):
            cents_mm[:, 8 * c + a, c * K : (c + 1) * K] = cs[c, :, a, :].T
    cmm = np.ascontiguousarray(cents_mm.transpose(1, 0, 2).reshape(CIN, 9 * CK))

    c2 = (centroids * centroids).sum(-1).reshape(CK)  # [NC*K]
    c2g = np.ascontiguousarray(
        np.broadcast_to((-0.5 * c2)[None, :], (CIN, CK))
    ).astype(np.float32)

    # lut[c*K+k, o] = (centroids[c] @ weight[c])[k, o] + bias[o]/NC
    lut_full = np.einsum("cks,cso->cko", centroids, weight).reshape(CK, COUT)
    lut_full = lut_full + bias[None, :] / NC
    lut2 = np.concatenate([lut_full[:CIN], lut_full[CIN:]], axis=1)  # [128, 512]
    lut2 = np.ascontiguousarray(lut2).astype(np.float32)
    return cmm, c2g, lut2


def prep_x(xi):
    xp = np.zeros((CIN, XPL), dtype=np.float32)
    xp[:, 1 : 1 + PW * PW] = np.pad(xi, ((0, 0), (1, 1), (1, 1))).reshape(
        CIN, PW * PW
    )
    return xp


def prep_in_maps(x, centroids, weight, bias):
    x = np.asarray(x, dtype=np.float32)
    cmm, c2g, lut2 = prep_consts(centroids, weight, bias)
    idn = np.eye(NT, dtype=np.float32)
    return [
        {
            "xp": prep_x(x[i]),
            "cmm": cmm,
            "c2g": c2g,
            "lut": lut2,
            "idn": idn,
        }
        for i in range(B)
    ]


# valid-position selector over the 26*128 flat slots
_PFLAT = np.arange(P0, P0 + NTILES * NT)
_PSEL = (_PFLAT <= PLAST) & (_PFLAT % PW >= 1) & (_PFLAT % PW <= W)


def unpack_out(raw):
    """raw [CIN, NTILES*COUT] f16 -> [COUT, H, W] f32 for one image."""
    arr = np.asarray(raw, dtype=np.float32).reshape(CIN, NTILES, COUT)
    a = arr.transpose(1, 0, 2).reshape(NTILES * NT, COUT)  # flat slot-major
    return a[_PSEL].reshape(H, W, COUT).transpose(2, 0, 1)


_NC_CACHE = []


def kernel(x, centroids, weight, inverse_temperature_logit, bias):
    if not _NC_CACHE:
        _NC_CACHE.append(build())
    nc = _NC_CACHE[0]

    in_maps = prep_in_maps(x, centroids, weight, bias)
    res = run_bass_kernel_spmd(nc, in_maps, core_ids=list(range(B)))
    out = np.stack([unpack_out(res.results[i]["out"]) for i in range(B)])
    return np.ascontiguousarray(out.astype(np.float32))


# revision 13
# speedup vs baseline: 1.0645x; 1.0645x over previous
import sys

sys.path.insert(0, "/opt/trn_rl_repo")

import numpy as np

import concourse.bacc as bacc
import concourse.mybir as mybir
import concourse.tile as tile
from concourse.bass_utils import run_bass_kernel_spmd
from concourse.masks import make_identity

# Problem constants (nn_AMMConv2d: 3x3 conv via product quantization, STE forward)
NC, K, SUB = 16, 16, 72
CIN, COUT = 128, 256
H = W = 56
B = 8
PW = W + 2             # padded width 58
NT = 128               # positions per tile (flattened padded coords)
P0 = PW + 1            # first valid flat position (row 1, col 1) = 59
PLAST = W * PW + W     # last valid flat position = 3304
NTILES = (PLAST - P0 + 1 + NT - 1) // NT   # 26
XPL = 3456             # 1 guard col + 58*58 padded image + tail guard
CK = NC * K            # 256
# tap offsets in flattened padded coords (kh-1)*PW + (kw-1)
TAPS = [(kh - 1) * PW + (kw - 1) for kh in range(3) for kw in range(3)]
# xp DMA chunks in columns (first small for fast pipeline start)
XCHUNKS = [256, 512, 768, 896, 1024]
# output tile groups (start, size); tail groups shrink to drain faster
_OG = [(0, 4), (4, 4), (8, 4), (12, 4), (16, 4), (20, 2), (22, 2), (24, 1), (25, 1)]
OGROUPS = {}
for _s, _n in _OG:
    for _t in range(_s, _s + _n):
        OGROUPS[_t] = (_s, _n)
OGRP = 4
NWARM = 12             # PE warmup transposes (p-state ramp while DMAs land)

F32 = mybir.dt.float32
F32R = mybir.dt.float32r
F16 = mybir.dt.float16


def build():
    nc = bacc.Bacc("TRN2", target_bir_lowering=False, debug=False)

    xp_ext = nc.declare_dram_parameter("xp", [CIN, XPL], F32R, isOutput=False)
    cmm_ext = nc.declare_dram_parameter("cmm", [CIN, 9 * CK], F32R, isOutput=False)
    c2g_ext = nc.declare_dram_parameter("c2g", [CIN, CK], F32, isOutput=False)
    lut_ext = nc.declare_dram_parameter("lut", [CIN, 2 * COUT], F16, isOutput=False)
    idn_ext = nc.declare_dram_parameter("idn", [NT, NT], F16, isOutput=False)
    # partition-major output: out[p, t, o]; host untangles valid rows
    out_ext = nc.declare_dram_parameter(
        "out", [CIN, NTILES * COUT], F16, isOutput=True
    )
    out3 = out_ext[:].rearrange("p (t o) -> p t o", t=NTILES)

    with tile.TileContext(nc) as tc:
        with (
            tc.tile_pool(name="const", bufs=1) as const_pool,
            tc.tile_pool(name="work", bufs=4) as work,
            tc.tile_pool(name="obuf", bufs=2) as obuf,
            tc.tile_pool(name="spsum", bufs=3, space="PSUM") as spsum,
            tc.tile_pool(name="tpsum", bufs=2, space="PSUM") as tpsum,
            tc.tile_pool(name="wpsum", bufs=1, space="PSUM") as wpsum,
            tc.tile_pool(name="opsum", bufs=2, space="PSUM") as opsum,
        ):
            xp = const_pool.tile([CIN, XPL], F32R)
            cmm = const_pool.tile([CIN, 9, CK], F32R)
            c2g = const_pool.tile([CIN, CK], F32)
            lut = const_pool.tile([CIN, 2, COUT], F16)

            # identity first so PE warmup can start immediately
            ident = const_pool.tile([NT, NT], F16)

            # sync queue: xp column chunks, small first
            c0 = 0
            for ncols in XCHUNKS:
                c1 = min(c0 + ncols, XPL)
                nc.sync.dma_start(xp[:, c0:c1], xp_ext[:, c0:c1])
                c0 = c1
            # centroid matrices: tap pairs split across two queues for
            # parallel issue + transfer (per-tap sems stagger availability)
            nc.scalar.dma_start(ident[:], idn_ext[:])
            cmm2 = cmm[:].rearrange("p a b -> p (a b)")
            for lo, hi in ((0, 1), (1, 3), (3, 5)):
                nc.scalar.dma_start(
                    cmm2[:, lo * CK : hi * CK], cmm_ext[:, lo * CK : hi * CK]
                )
            for lo, hi in ((5, 7), (7, 9)):
                nc.gpsimd.dma_start(
                    cmm2[:, lo * CK : hi * CK], cmm_ext[:, lo * CK : hi * CK]
                )
            # epilogue consts (needed ~1 tile in)
            nc.gpsimd.dma_start(c2g[:], c2g_ext[:])
            nc.gpsimd.dma_start(
                lut[:].rearrange("p a b -> p (a b)"), lut_ext[:]
            )

            # PE warmup: ramp the tensor-engine p-state while DMAs land
            warm_ps = wpsum.tile([NT, NT], F16, tag="warm", name="warm")
            for _ in range(NWARM):
                nc.tensor.transpose(warm_ps[:], ident[:], ident[:])

            # ---------- main loop: 3-deep software pipeline ----------
            def emit_scores(t):
                base = 1 + P0 + t * NT  # guard col + flat window start
                s_ps = spsum.tile([NT, CK], F32, tag="scores", name="s_ps")
                for kk in range(9):
                    off = TAPS[kk]
                    nc.tensor.matmul(
                        s_ps[:],
                        xp[:, base + off : base + off + NT],
                        cmm[:, kk, :],
                        start=(kk == 0),
                        stop=(kk == 8),
                    )
                return s_ps

            def emit_epi_a(t, s_ps):
                # g = xc - c2/2 (gpsimd); row-max per codebook + one-hot (DVE)
                g = work.tile([NT, CK], F32, tag="g", name="g")
                nc.vector.tensor_tensor(
                    g[:], s_ps[:], c2g[:], mybir.AluOpType.add
                )
                g3 = g[:].rearrange("p (c k) -> p c k", c=NC)
                gmax = work.tile([NT, NC], F32, tag="gmax", name="gmax")
                nc.vector.tensor_reduce(
                    gmax[:], g3, axis=mybir.AxisListType.X, op=mybir.AluOpType.max
                )
                mask = work.tile([NT, CK], F16, tag="mask", name="mask")
                nc.vector.tensor_tensor(
                    mask[:].rearrange("p (c k) -> p c k", c=NC),
                    g3,
                    gmax[:].unsqueeze(2).broadcast_to([NT, NC, K]),
                    mybir.AluOpType.is_equal,
                )
                return mask

            def emit_epi_b(t, mask, o_sb):
                mt_ps = tpsum.tile([CIN, 2 * NT], F16, tag="mt", name="mt_ps")
                for j in range(2):
                    nc.tensor.transpose(
                        mt_ps[:, j * NT : (j + 1) * NT],
                        mask[:, j * CIN : (j + 1) * CIN],
                        ident[:],
                    )
                mT = work.tile([CIN, 2 * NT], F16, tag="mT", name="mT")
                nc.scalar.activation(
                    mT[:], mt_ps[:], mybir.ActivationFunctionType.Copy
                )
                o_ps = opsum.tile([NT, COUT], F32, tag="out", name="o_ps")
                for j in range(2):
                    nc.tensor.matmul(
                        o_ps[:],
                        mT[:, j * NT : (j + 1) * NT],
                        lut[:, j, :],
                        start=(j == 0),
                        stop=(j == 1),
                    )
                t0, gsz = OGROUPS[t]
                slot = t - t0
                nc.scalar.activation(
                    o_sb[:, slot * COUT : (slot + 1) * COUT],
                    o_ps[:],
                    mybir.ActivationFunctionType.Copy,
                )
                if slot == gsz - 1:
                    nc.sync.dma_start(
                        out3[:, t0 : t0 + gsz, :],
                        o_sb[:, : gsz * COUT].rearrange(
                            "p (a b) -> p a b", a=gsz
                        ),
                    )

            def get_osb(t):
                if OGROUPS[t][0] == t:
                    get_osb.cur = obuf.tile(
                        [NT, OGRP * COUT], F16, tag="osb", name="o_sb"
                    )
                return get_osb.cur

            stage_a = []  # (t, s_ps)
            stage_b = []  # (t, mask)
            for t in range(NTILES):
                stage_a.append((t, emit_scores(t)))
                if len(stage_a) > 1:
                    ta, s_ps = stage_a.pop(0)
                    stage_b.append((ta, emit_epi_a(ta, s_ps)))
                if len(stage_b) > 1:
                    tb, mask = stage_b.pop(0)
                    emit_epi_b(tb, mask, get_osb(tb))
            while stage_a:
                ta, s_ps = stage_a.pop(0)
                stage_b.append((ta, emit_epi_a(ta, s_ps)))
            while stage_b:
                tb, mask = stage_b.pop(0)
                emit_epi_b(tb, mask, get_osb(tb))

    nc.compile()
    return nc


def prep_consts(centroids, weight, bias):
    """Host-side constant packing (exact f32/f16; no device prologue math)."""
    centroids = np.asarray(centroids, dtype=np.float32)
    weight = np.asarray(weight, dtype=np.float32)
    bias = np.asarray(bias, dtype=np.float32)

    # cmm[8c+a, kk*CK + c*K + k] = centroids[c, k, a*9 + kk]
    cents_mm = np.zeros((9, CIN, CK), dtype=np.float32)
    cs = centroids.reshape(NC, K, 8, 9)  # s = a*9 + kk
    for c in range(NC):
        for a in range(8):
            cents_mm[:, 8 * c + a, c * K : (c + 1) * K] = cs[c, :, a, :].T
    cmm = np.ascontiguousarray(cents_mm.transpose(1, 0, 2).reshape(CIN, 9 * CK))

    c2 = (centroids * centroids).sum(-1).reshape(CK)  # [NC*K]
    c2g = np.ascontiguousarray(
        np.broadcast_to((-0.5 * c2)[None, :], (CIN, CK))
    ).astype(np.float32)

    # lut[c*K+k, o] = (centroids[c] @ weight[c])[k, o] + bias[o]/NC
    lut_full = np.einsum("cks,cso->cko", centroids, weight).reshape(CK, COUT)
    lut_full = lut_full + bias[None, :] / NC
    lut2 = np.concatenate([lut_full[:CIN], lut_full[CIN:]], axis=1)  # [128, 512]
    lut2 = np.ascontiguousarray(lut2).astype(np.float16)
    return cmm, c2g, lut2


def prep_x(xi):
    xp = np.zeros((CIN, XPL), dtype=np.float32)
    xp[:, 1 : 1 + PW * PW] = np.pad(xi, ((0, 0), (1, 1), (1, 1))).reshape(
        CIN, PW * PW
    )
    return xp


def prep_in_maps(x, centroids, weight, bias):
    x = np.asarray(x, dtype=np.float32)
    cmm, c2g, lut2 = prep_consts(centroids, weight, bias)
    idn = np.eye(NT, dtype=np.float16)
    return [
        {
            "xp": prep_x(x[i]),
            "cmm": cmm,
            "c2g": c2g,
            "lut": lut2,
            "idn": idn,
        }
        for i in range(B)
    ]


# valid-position selector over the 26*128 flat slots
_PFLAT = np.arange(P0, P0 + NTILES * NT)
_PSEL = (_PFLAT <= PLAST) & (_PFLAT % PW >= 1) & (_PFLAT % PW <= W)


def unpack_out(raw):
    """raw [CIN, NTILES*COUT] f16 -> [COUT, H, W] f32 for one image."""
    arr = np.asarray(raw, dtype=np.float32).reshape(CIN, NTILES, COUT)
    a = arr.transpose(1, 0, 2).reshape(NTILES * NT, COUT)  # flat slot-major
    return a[_PSEL].reshape(H, W, COUT).transpose(2, 0, 1)


_NC_CACHE = []


def kernel(x, centroids, weight, inverse_temperature_logit, bias):
    if not _NC_CACHE:
        _NC_CACHE.append(build())
    nc = _NC_CACHE[0]

    in_maps = prep_in_maps(x, centroids, weight, bias)
    res = run_bass_kernel_spmd(nc, in_maps, core_ids=list(range(B)))
    out = np.stack([unpack_out(res.results[i]["out"]) for i in range(B)])
    return np.ascontiguousarray(out.astype(np.float32))


# revision 14
# speedup vs baseline: 1.1659x; 1.0953x over previous
import sys

sys.path.insert(0, "/opt/trn_rl_repo")

import numpy as np

import concourse.bacc as bacc
import concourse.mybir as mybir
import concourse.tile as tile
from concourse.bass_utils import run_bass_kernel_spmd
from concourse.masks import make_identity

# Problem constants (nn_AMMConv2d: 3x3 conv via product quantization, STE forward)
NC, K, SUB = 16, 16, 72
CIN, COUT = 128, 256
H = W = 56
B = 8
PW = W + 2             # padded width 58
NT = 128               # positions per tile (flattened padded coords)
P0 = PW + 1            # first valid flat position (row 1, col 1) = 59
PLAST = W * PW + W     # last valid flat position = 3304
NTILES = (PLAST - P0 + 1 + NT - 1) // NT   # 26
XPL = 3456             # 1 guard col + 58*58 padded image + tail guard
CK = NC * K            # 256
# tap offsets in flattened padded coords (kh-1)*PW + (kw-1)
TAPS = [(kh - 1) * PW + (kw - 1) for kh in range(3) for kw in range(3)]
# xp DMA chunks in columns (first small for fast pipeline start)
XCHUNKS = [256, 512, 768, 896, 1024]
# output tile groups (start, size); tail groups shrink to drain faster
_OG = [(0, 4), (4, 4), (8, 4), (12, 4), (16, 4), (20, 2), (22, 2), (24, 1), (25, 1)]
OGROUPS = {}
for _s, _n in _OG:
    for _t in range(_s, _s + _n):
        OGROUPS[_t] = (_s, _n)
OGRP = 4
NWARM = 12             # PE warmup transposes (p-state ramp while DMAs land)

F32 = mybir.dt.float32
F32R = mybir.dt.float32r
F16 = mybir.dt.float16


def build():
    nc = bacc.Bacc("TRN2", target_bir_lowering=False, debug=False)

    xp_ext = nc.declare_dram_parameter("xp", [CIN, XPL], F32R, isOutput=False)
    cmm_ext = nc.declare_dram_parameter("cmm", [CIN, 9 * CK], F32R, isOutput=False)
    c2g_ext = nc.declare_dram_parameter("c2g", [CIN, CK], F32, isOutput=False)
    lut_ext = nc.declare_dram_parameter("lut", [CIN, 2 * COUT], F16, isOutput=False)
    idn_ext = nc.declare_dram_parameter("idn", [NT, NT], F16, isOutput=False)
    # partition-major output: out[p, t, o]; host untangles valid rows
    out_ext = nc.declare_dram_parameter(
        "out", [CIN, NTILES * COUT], F16, isOutput=True
    )
    out3 = out_ext[:].rearrange("p (t o) -> p t o", t=NTILES)

    with tile.TileContext(nc) as tc:
        with (
            tc.tile_pool(name="const", bufs=1) as const_pool,
            tc.tile_pool(name="work", bufs=4) as work,
            tc.tile_pool(name="obuf", bufs=3) as obuf,
            tc.tile_pool(name="spsum", bufs=3, space="PSUM") as spsum,
            tc.tile_pool(name="tpsum", bufs=2, space="PSUM") as tpsum,
            tc.tile_pool(name="wpsum", bufs=1, space="PSUM") as wpsum,
            tc.tile_pool(name="opsum", bufs=2, space="PSUM") as opsum,
        ):
            xp = const_pool.tile([CIN, XPL], F32R)
            cmm = const_pool.tile([CIN, 9, CK], F32R)
            c2g = const_pool.tile([CIN, CK], F32)
            lut = const_pool.tile([CIN, 2, COUT], F16)

            # identity first so PE warmup can start immediately
            ident = const_pool.tile([NT, NT], F16)

            # sync queue: xp column chunks, small first; chunks 1+ overlap
            # the previous chunk by one column so they chain (WAW dep) and
            # leave early DMA bandwidth to the critical cmm transfers
            c0 = 0
            for ncols in XCHUNKS:
                c1 = min(c0 + ncols, XPL)
                lo = max(0, c0 - 1)
                nc.sync.dma_start(xp[:, lo:c1], xp_ext[:, lo:c1])
                c0 = c1
            # centroid matrices: tap pairs split across two queues for
            # parallel issue + transfer (per-tap sems stagger availability)
            nc.scalar.dma_start(ident[:], idn_ext[:])
            cmm2 = cmm[:].rearrange("p a b -> p (a b)")
            for lo, hi in ((0, 1), (1, 3), (3, 5)):
                nc.scalar.dma_start(
                    cmm2[:, lo * CK : hi * CK], cmm_ext[:, lo * CK : hi * CK]
                )
            for lo, hi in ((5, 7), (7, 9)):
                nc.gpsimd.dma_start(
                    cmm2[:, lo * CK : hi * CK], cmm_ext[:, lo * CK : hi * CK]
                )
            # epilogue consts (needed ~1 tile in)
            nc.gpsimd.dma_start(c2g[:], c2g_ext[:])
            nc.gpsimd.dma_start(
                lut[:].rearrange("p a b -> p (a b)"), lut_ext[:]
            )

            # PE warmup: ramp the tensor-engine p-state while DMAs land
            warm_ps = wpsum.tile([NT, NT], F16, tag="warm", name="warm")
            for _ in range(NWARM):
                nc.tensor.transpose(warm_ps[:], ident[:], ident[:])

            # ---------- main loop: 3-deep software pipeline ----------
            def emit_scores(t):
                base = 1 + P0 + t * NT  # guard col + flat window start
                s_ps = spsum.tile([NT, CK], F32, tag="scores", name="s_ps")
                for kk in range(9):
                    off = TAPS[kk]
                    nc.tensor.matmul(
                        s_ps[:],
                        xp[:, base + off : base + off + NT],
                        cmm[:, kk, :],
                        start=(kk == 0),
                        stop=(kk == 8),
                    )
                return s_ps

            def emit_epi_a(t, s_ps):
                # g = xc - c2/2 (gpsimd); row-max per codebook + one-hot (DVE)
                g = work.tile([NT, CK], F32, tag="g", name="g")
                nc.vector.tensor_tensor(
                    g[:], s_ps[:], c2g[:], mybir.AluOpType.add
                )
                g3 = g[:].rearrange("p (c k) -> p c k", c=NC)
                gmax = work.tile([NT, NC], F32, tag="gmax", name="gmax")
                nc.vector.tensor_reduce(
                    gmax[:], g3, axis=mybir.AxisListType.X, op=mybir.AluOpType.max
                )
                mask = work.tile([NT, CK], F16, tag="mask", name="mask")
                nc.vector.tensor_tensor(
                    mask[:].rearrange("p (c k) -> p c k", c=NC),
                    g3,
                    gmax[:].unsqueeze(2).broadcast_to([NT, NC, K]),
                    mybir.AluOpType.is_equal,
                )
                return mask

            def emit_epi_b1(t, mask):
                mt_ps = tpsum.tile([CIN, 2 * NT], F16, tag="mt", name="mt_ps")
                for j in range(2):
                    nc.tensor.transpose(
                        mt_ps[:, j * NT : (j + 1) * NT],
                        mask[:, j * CIN : (j + 1) * CIN],
                        ident[:],
                    )
                mT = work.tile([CIN, 2 * NT], F16, tag="mT", name="mT")
                nc.scalar.activation(
                    mT[:], mt_ps[:], mybir.ActivationFunctionType.Copy
                )
                return mT

            def emit_epi_b2(t, mT, o_sb):
                o_ps = opsum.tile([NT, COUT], F32, tag="out", name="o_ps")
                for j in range(2):
                    nc.tensor.matmul(
                        o_ps[:],
                        mT[:, j * NT : (j + 1) * NT],
                        lut[:, j, :],
                        start=(j == 0),
                        stop=(j == 1),
                    )
                t0, gsz = OGROUPS[t]
                slot = t - t0
                nc.scalar.activation(
                    o_sb[:, slot * COUT : (slot + 1) * COUT],
                    o_ps[:],
                    mybir.ActivationFunctionType.Copy,
                )
                if slot == gsz - 1:
                    nc.sync.dma_start(
                        out3[:, t0 : t0 + gsz, :],
                        o_sb[:, : gsz * COUT].rearrange(
                            "p (a b) -> p a b", a=gsz
                        ),
                    )

            def get_osb(t):
                if OGROUPS[t][0] == t:
                    get_osb.cur = obuf.tile(
                        [NT, OGRP * COUT], F16, tag="osb", name="o_sb"
                    )
                return get_osb.cur

            stage_a = []  # (t, s_ps)
            stage_b = []  # (t, mask)
            stage_c = []  # (t, mT)
            def pump(drain=False):
                if len(stage_a) > (0 if drain else 1):
                    ta, s_ps = stage_a.pop(0)
                    stage_b.append((ta, emit_epi_a(ta, s_ps)))
                if len(stage_b) > (0 if drain else 1):
                    tb, mask = stage_b.pop(0)
                    stage_c.append((tb, emit_epi_b1(tb, mask)))
                if len(stage_c) > (0 if drain else 1):
                    tc_, mT = stage_c.pop(0)
                    emit_epi_b2(tc_, mT, get_osb(tc_))
            for t in range(NTILES):
                stage_a.append((t, emit_scores(t)))
                pump()
            while stage_a or stage_b or stage_c:
                pump(drain=True)

    nc.compile()
    return nc


def prep_consts(centroids, weight, bias):
    """Host-side constant packing (exact f32/f16; no device prologue math)."""
    centroids = np.asarray(centroids, dtype=np.float32)
    weight = np.asarray(weight, dtype=np.float32)
    bias = np.asarray(bias, dtype=np.float32)

    # cmm[8c+a, kk*CK + c*K + k] = centroids[c, k, a*9 + kk]
    cents_mm = np.zeros((9, CIN, CK), dtype=np.float32)
    cs = centroids.reshape(NC, K, 8, 9)  # s = a*9 + kk
    for c in range(NC):
        for a in range(8):
            cents_mm[:, 8 * c + a, c * K : (c + 1) * K] = cs[c, :, a, :].T
    cmm = np.ascontiguousarray(cents_mm.transpose(1, 0, 2).reshape(CIN, 9 * CK))

    c2 = (centroids * centroids).sum(-1).reshape(CK)  # [NC*K]
    c2g = np.ascontiguousarray(
        np.broadcast_to((-0.5 * c2)[None, :], (CIN, CK))
    ).astype(np.float32)

    # lut[c*K+k, o] = (centroids[c] @ weight[c])[k, o] + bias[o]/NC
    lut_full = np.einsum("cks,cso->cko", centroids, weight).reshape(CK, COUT)
    lut_full = lut_full + bias[None, :] / NC
    lut2 = np.concatenate([lut_full[:CIN], lut_full[CIN:]], axis=1)  # [128, 512]
    lut2 = np.ascontiguousarray(lut2).astype(np.float16)
    return cmm, c2g, lut2


def prep_x(xi):
    xp = np.zeros((CIN, XPL), dtype=np.float32)
    xp[:, 1 : 1 + PW * PW] = np.pad(xi, ((0, 0), (1, 1), (1, 1))).reshape(
        CIN, PW * PW
    )
    return xp


def prep_in_maps(x, centroids, weight, bias):
    x = np.asarray(x, dtype=np.float32)
    cmm, c2g, lut2 = prep_consts(centroids, weight, bias)
    idn = np.eye(NT, dtype=np.float16)
    return [
        {
            "xp": prep_x(x[i]),
            "cmm": cmm,
            "c2g": c2g,
            "lut": lut2,
            "idn": idn,
        }
        for i in range(B)
    ]


# valid-position selector over the 26*128 flat slots
_PFLAT = np.arange(P0, P0 + NTILES * NT)
_PSEL = (_PFLAT <= PLAST) & (_PFLAT % PW >= 1) & (_PFLAT % PW <= W)


def unpack_out(raw):
    """raw [CIN, NTILES*COUT] f16 -> [COUT, H, W] f32 for one image."""
    arr = np.asarray(raw, dtype=np.float32).reshape(CIN, NTILES, COUT)
    a = arr.transpose(1, 0, 2).reshape(NTILES * NT, COUT)  # flat slot-major
    return a[_PSEL].reshape(H, W, COUT).transpose(2, 0, 1)


_NC_CACHE = []


def kernel(x, centroids, weight, inverse_temperature_logit, bias):
    if not _NC_CACHE:
        _NC_CACHE.append(build())
    nc = _NC_CACHE[0]

    in_maps = prep_in_maps(x, centroids, weight, bias)
    res = run_bass_kernel_spmd(nc, in_maps, core_ids=list(range(B)))
    out = np.stack([unpack_out(res.results[i]["out"]) for i in range(B)])
    return np.ascontiguousarray(out.astype(np.float32))
